# revision 32
# baseline (speedup 1.0000x reference)
"""Trainium2 Bass kernel for nn_BoxAccuracy (nms_detection).

Computes, fully on-device (replicated across the 8 NeuronCores — degenerate
data-parallel over the single detection frame, per the sharding hint):
  - target box selection: first-k nonzero target confidences (lexicographic)
  - pred box selection: p_conf >= (k-th largest p_conf), first k lexicographic
  - per-box decode (trunc to int coords), pairwise IoU [k,k],
    mean over target boxes of max IoU vs pred boxes.

Host side only RE-ARRANGES raw input values (chunk layout + confidence row
replicated across partitions + constant gx/gy tables); all arithmetic
(scaling, rank counts, cumsum compaction, decode, IoU) runs on device.

Device algorithm:
  * 640 candidate cells as 5 chunks x 128 partitions, linear i = c*128 + p.
  * pred mask via rank counts (count_{j: c_j > c_i} < k  <=>  conf_i >=
    k-th largest, tie-exact): 5 tensor_scalar(is_gt, accum_out) ops at 2x
    mode against the replicated conf row.
  * first-k compaction: inclusive cumsum = L^T @ mask + allones^T @ mprefix
    (one-hot S = (cum == iota) * mask), then gathers S^T @ V on the PE.
  * trunc built from the f32->i32 convert (rounding-mode-agnostic
    correction: t = r - sgn*((r-v)*sgn > 0)).
  * t-side pipeline on GPSIMD/PE/ACT overlaps the p-side critical path
    (counts -> cumsum -> gather -> decode -> IoU) on the DVE.
  * p-coords row-broadcast via diag(col) matmuls with an all-ones stationary.
"""

import sys

if "/opt/trn_rl_repo" not in sys.path:
    sys.path.insert(0, "/opt/trn_rl_repo")

import numpy as np

C = 190
HW = 64  # 8x8 grid flattened
NB = 10  # boxes (conf channels)
NCELL = NB * HW  # 640
NCHUNK = 5  # 640 / 128
P = 128
CELLX = 240.0
CELLY = 151.0

_CACHE = {}


def _host_arrange(arr):
    """[190, 64] -> V chunk layout [128, 5, 7]: partition p = (a%2)*64 + xy,
    chunk c = a//2; cols = conf, ox, oy, ow, oh, gx*CELLX, gy*CELLY.
    Pure gather of raw input values plus constant cell tables."""
    v = np.zeros((P, NCHUNK, 7), np.float32)
    p = np.arange(P)
    xy = p % 64
    for c in range(NCHUNK):
        a = 2 * c + (p >= 64)
        base = a * 19
        for r in range(5):
            v[:, c, r] = arr[base + r, xy]
    v[:, :, 5] = (xy // 8).astype(np.float32)[:, None]
    v[:, :, 6] = (xy % 8).astype(np.float32)[:, None]
    return v


def build(k):
    """Build the Bass program for a given (python int) k. Returns nc."""
    assert 25 <= k <= 128, f"k={k} unsupported (need 25..128)"
    import concourse.bacc as bacc
    import concourse.mybir as mybir
    from concourse.tile import TileContext

    AO = mybir.AluOpType
    f32 = mybir.dt.float32
    i32 = mybir.dt.int32

    nc = bacc.Bacc("TRN2", target_bir_lowering=False, debug=False)

    vp_d = nc.dram_tensor("vp", [P, NCHUNK * 7], f32, kind="ExternalInput")
    vt_d = nc.dram_tensor("vt", [P, NCHUNK * 7], f32, kind="ExternalInput")
    rba_d = nc.dram_tensor("rba", [P, NCELL // 2], f32, kind="ExternalInput")
    rbb_d = nc.dram_tensor("rbb", [P, NCELL // 2], f32, kind="ExternalInput")
    out_d = nc.dram_tensor("out", [1, 1], f32, kind="ExternalOutput")

    with TileContext(nc) as tc:
        with (
            tc.sbuf_pool(name="sb", bufs=1) as sb,
            tc.psum_pool(name="ps", bufs=1) as ps,
        ):
            # ---------------- SBUF tiles ----------------
            Vp = sb.tile([P, NCHUNK, 7], f32)  # conf ox oy ow oh gx gy
            Vt = sb.tile([P, NCHUNK, 7], f32)
            rba_sb = sb.tile([P, NCELL // 2], f32)
            rbb_sb = sb.tile([P, NCELL // 2], f32)
            ident = sb.tile([P, P], f32)
            allones = sb.tile([P, P], f32)
            iota = sb.tile([P, k], f32)
            L = sb.tile([P, P], f32)  # L[p, m] = 1 if p <= m
            ones_col = sb.tile([P, 1], f32)
            cell2 = sb.tile([P, 2], f32)  # CX, CY
            cellh2 = sb.tile([P, 2], f32)  # CX/2, CY/2
            V4p = sb.tile([P, NCHUNK, 4], f32)
            V4t = sb.tile([P, NCHUNK, 4], f32)
            vs_p = sb.tile([P, NCHUNK, 2], f32)
            vs_t = sb.tile([P, NCHUNK, 2], f32)
            vc_p = sb.tile([P, NCHUNK, 2], f32)
            vc_t = sb.tile([P, NCHUNK, 2], f32)
            vw_p = sb.tile([P, NCHUNK, 2], f32)
            vw_t = sb.tile([P, NCHUNK, 2], f32)
            junk = sb.tile([P, NCELL // 2], f32)
            cnt_a = sb.tile([P, NCHUNK], f32)
            cnt_b = sb.tile([P, NCHUNK], f32)
            cnt = sb.tile([P, NCHUNK], f32)
            masks_t = sb.tile([P, NCHUNK], f32)
            masks_p = sb.tile([P, NCHUNK], f32)
            cum_t_sb = sb.tile([P, NCHUNK], f32)
            cum_p_sb = sb.tile([P, NCHUNK], f32)
            S_t = sb.tile([P, NCHUNK, k], f32)
            S_p = sb.tile([P, NCHUNK, k], f32)
            g_t_sb = sb.tile([k, 4], f32)
            ca_t = sb.tile([k, 5], f32)  # x1 y1 x2 y2 area
            ca_p = sb.tile([k, 5], f32)
            ti_t = sb.tile([k, 4], i32)
            ti_p = sb.tile([k, 4], i32)
            rr_t = sb.tile([k, 4], f32)
            rr_p = sb.tile([k, 4], f32)
            ee_t = sb.tile([k, 4], f32)
            ee_p = sb.tile([k, 4], f32)
            sg_t = sb.tile([k, 4], f32)
            sg_p = sb.tile([k, 4], f32)
            uu_t = sb.tile([k, 4], f32)
            uu_p = sb.tile([k, 4], f32)
            mm_t = sb.tile([k, 4], f32)
            mm_p = sb.tile([k, 4], f32)
            gg_t = sb.tile([k, 4], f32)
            dxy_t = sb.tile([k, 2], f32)
            dxy_p = sb.tile([k, 2], f32)
            diag = sb.tile([k, 5, k], f32)
            lt2 = sb.tile([k, 2 * k], f32)
            rb2 = sb.tile([k, 2 * k], f32)
            wh = sb.tile([k, 2 * k], f32)
            whr_y = sb.tile([k, k], f32)
            inter = sb.tile([k, k], f32)
            union = sb.tile([k, k], f32)
            recip = sb.tile([k, k], f32)
            iou = sb.tile([k, k], f32)
            rowmax = sb.tile([k, 1], f32)
            out_sb = sb.tile([1, 1], f32)

            # ---------------- input DMAs ----------------
            nc.sync.dma_start(out=rbb_sb, in_=rbb_d.ap())
            nc.sync.dma_start(
                out=Vp.rearrange("p c r -> p (c r)"), in_=vp_d.ap()
            )
            nc.sync.dma_start(out=rba_sb, in_=rba_d.ap())
            nc.sync.dma_start(
                out=Vt.rearrange("p c r -> p (c r)"), in_=vt_d.ap()
            )

            # ---------------- on-device constants ----------------
            nc.vector.memset(ones_col, 1.0)
            for col, val in enumerate([CELLX, CELLY]):
                nc.gpsimd.memset(cell2[:, col : col + 1], val)
                nc.gpsimd.memset(cellh2[:, col : col + 1], val / 2)
            nc.gpsimd.memset(ident, 0.0)
            nc.gpsimd.affine_select(
                out=ident, in_=ident, compare_op=AO.not_equal, fill=1.0,
                base=0, pattern=[[-1, P]], channel_multiplier=1,
            )
            # L[p, m] = 1 iff m - p >= 0
            nc.gpsimd.memset(L, 1.0)
            nc.gpsimd.affine_select(
                out=L, in_=L, compare_op=AO.is_ge, fill=0.0,
                base=0, pattern=[[1, P]], channel_multiplier=-1,
            )
            nc.gpsimd.iota(
                iota, pattern=[[1, k]], base=1, channel_multiplier=0,
                allow_small_or_imprecise_dtypes=True,
            )
            nc.gpsimd.memset(allones, 1.0)

            cell_b = cell2.unsqueeze(1).to_broadcast([P, NCHUNK, 2])
            cellh_b = cellh2.unsqueeze(1).to_broadcast([P, NCHUNK, 2])

            def build_v4(eng, V, V4, vs, vc, vw):
                # pre-gather corner columns: cen = (o + cell)*CELL,
                # corners = cen -/+ (wh*CELL/2)  (linear in V columns)
                eng.tensor_tensor(vs, V[:, :, 1:3], V[:, :, 5:7], op=AO.add)
                eng.tensor_tensor(vc, vs, cell_b, op=AO.mult)
                eng.tensor_tensor(vw, V[:, :, 3:5], cellh_b, op=AO.mult)
                eng.tensor_tensor(V4[:, :, 0:2], vc, vw, op=AO.subtract)
                eng.tensor_tensor(V4[:, :, 2:4], vc, vw, op=AO.add)

            build_v4(nc.vector, Vp, V4p, vs_p, vc_p, vw_p)

            # ---------------- rank counts -> mask_p (DVE critical) --------
            # two half-row passes, pipelined against the two rb DMAs
            for c in range(NCHUNK):
                nc.vector.tensor_scalar(
                    junk, rbb_sb, Vp[:, c, 0:1], None,
                    op0=AO.is_gt, op1=AO.add, accum_out=cnt_b[:, c : c + 1],
                )
            for c in range(NCHUNK):
                nc.vector.tensor_scalar(
                    junk, rba_sb, Vp[:, c, 0:1], None,
                    op0=AO.is_gt, op1=AO.add, accum_out=cnt_a[:, c : c + 1],
                )
            nc.vector.tensor_tensor(cnt, cnt_a, cnt_b, op=AO.add)
            nc.vector.tensor_scalar(masks_p, cnt, float(k), None, op0=AO.is_lt)

            # ---------------- t-side (gpsimd/PE/ACT) ---------------------
            build_v4(nc.vector, Vt, V4t, vs_t, vc_t, vw_t)
            nc.vector.tensor_scalar(
                masks_t, Vt[:, :, 0], 0.0, None, op0=AO.not_equal
            )
            mpre_t = sb.tile([P, NCHUNK], f32)
            nc.vector.memset(mpre_t[:, 0:1], 0.0)
            nc.vector.tensor_copy(mpre_t[:, 1:2], masks_t[:, 0:1])
            for cc in range(2, NCHUNK):
                nc.vector.tensor_tensor(
                    mpre_t[:, cc : cc + 1], mpre_t[:, cc - 1 : cc],
                    masks_t[:, cc - 1 : cc], op=AO.add,
                )
            cum_t_ps = ps.tile([P, NCHUNK], f32, tag="cumt")
            nc.tensor.matmul(cum_t_ps, L, masks_t, start=True, stop=False)
            nc.tensor.matmul(cum_t_ps, allones, mpre_t, start=False, stop=True)
            nc.scalar.copy(cum_t_sb, cum_t_ps)
            for c in range(NCHUNK):
                nc.vector.tensor_scalar(
                    S_t[:, c, :], iota,
                    cum_t_sb[:, c : c + 1], masks_t[:, c : c + 1],
                    op0=AO.is_equal, op1=AO.mult,
                )
            g_t_ps = ps.tile([k, 4], f32, tag="gt")

            def decode(eng, g, ca, ti, rr, ee, sg, uu, mm_, dxy, gg=None):
                # g: pre-trunc corners [k, 4] (PSUM for DVE, SBUF for Pool)
                # trunc(v) = r - sgn * ((r - v) * sgn > 0), r = int-cast(v)
                eng.tensor_copy(ti, g)
                eng.tensor_copy(rr, ti)
                eng.tensor_scalar(sg, g, 0.0, 2.0, op0=AO.is_ge, op1=AO.mult)
                eng.tensor_scalar(sg, sg, 1.0, None, op0=AO.subtract)
                eng.tensor_tensor(ee, rr, g, op=AO.subtract)
                eng.tensor_tensor(uu, ee, sg, op=AO.mult)
                if gg is None:  # DVE: fused compare+mult
                    eng.scalar_tensor_tensor(
                        mm_, uu, 0.0, sg, op0=AO.is_gt, op1=AO.mult
                    )
                else:  # Pool has no scalar_tensor_tensor
                    eng.tensor_scalar(gg, uu, 0.0, None, op0=AO.is_gt)
                    eng.tensor_tensor(mm_, gg, sg, op=AO.mult)
                eng.tensor_tensor(ca[:, 0:4], rr, mm_, op=AO.subtract)
                eng.tensor_tensor(dxy, ca[:, 2:4], ca[:, 0:2], op=AO.subtract)
                eng.tensor_tensor(
                    ca[:, 4:5], dxy[:, 0:1], dxy[:, 1:2], op=AO.mult
                )

            # ---------------- p-side cumsum / S --------------------------
            mpre_p = sb.tile([P, NCHUNK], f32)
            nc.vector.memset(mpre_p[:, 0:1], 0.0)
            nc.vector.tensor_copy(mpre_p[:, 1:2], masks_p[:, 0:1])
            for cc in range(2, NCHUNK):
                nc.vector.tensor_tensor(
                    mpre_p[:, cc : cc + 1], mpre_p[:, cc - 1 : cc],
                    masks_p[:, cc - 1 : cc], op=AO.add,
                )
            cum_p_ps = ps.tile([P, NCHUNK], f32, tag="cump")
            nc.tensor.matmul(cum_p_ps, L, masks_p, start=True, stop=False)
            nc.tensor.matmul(cum_p_ps, allones, mpre_p, start=False, stop=True)
            nc.vector.tensor_copy(cum_p_sb, cum_p_ps)
            for c in range(NCHUNK):
                nc.vector.tensor_scalar(
                    S_p[:, c, :], iota,
                    cum_p_sb[:, c : c + 1], masks_p[:, c : c + 1],
                    op0=AO.is_equal, op1=AO.mult,
                )

            # ---------------- gathers + decode ----------------------------
            for c in range(NCHUNK):
                nc.tensor.matmul(
                    g_t_ps, S_t[:, c, :], V4t[:, c, :],
                    start=(c == 0), stop=(c == NCHUNK - 1),
                )
            nc.scalar.copy(g_t_sb, g_t_ps)
            decode(nc.vector, g_t_sb, ca_t, ti_t, rr_t, ee_t, sg_t,
                   uu_t, mm_t, dxy_t)

            g_p_ps = ps.tile([k, 4], f32, tag="gp")
            for c in range(NCHUNK):
                nc.tensor.matmul(
                    g_p_ps, S_p[:, c, :], V4p[:, c, :],
                    start=(c == 0), stop=(c == NCHUNK - 1),
                )
            decode(nc.vector, g_p_ps, ca_p, ti_p, rr_p, ee_p, sg_p,
                   uu_p, mm_p, dxy_p)

            # ------------- p rows broadcast: diag matmuls ---------------
            bc_lo = ps.tile([k, 2 * k], f32, tag="cumt")
            bc_hi = ps.tile([k, 3 * k], f32, tag="bchi")
            for r in range(5):
                nc.vector.tensor_scalar(
                    diag[:, r, :], ident[0:k, 0:k], ca_p[:, r : r + 1], None,
                    op0=AO.mult,
                )
                dst = (bc_lo[:, r * k : (r + 1) * k] if r < 2
                       else bc_hi[:, (r - 2) * k : (r - 1) * k])
                nc.tensor.matmul(
                    dst, allones[0:k, 0:k], diag[:, r, :],
                    skip_group_check=True,
                )

            # ---------------- IoU ----------------
            t_lo = ca_t[:, 0:2].unsqueeze(2).to_broadcast([k, 2, k])
            t_hi = ca_t[:, 2:4].unsqueeze(2).to_broadcast([k, 2, k])
            nc.vector.scalar_tensor_tensor(
                lt2.rearrange("p (b j) -> p b j", b=2),
                bc_lo.rearrange("p (b j) -> p b j", b=2), 0.0,
                t_lo, op0=AO.add, op1=AO.max,
            )
            nc.vector.scalar_tensor_tensor(
                rb2.rearrange("p (b j) -> p b j", b=2),
                bc_hi[:, 0 : 2 * k].rearrange("p (b j) -> p b j", b=2), 0.0,
                t_hi, op0=AO.add, op1=AO.min,
            )
            nc.vector.tensor_tensor(wh, rb2, lt2, op=AO.subtract)
            nc.vector.tensor_scalar(
                whr_y, wh[:, k : 2 * k], 0.0, None, op0=AO.max
            )
            nc.vector.scalar_tensor_tensor(
                inter, wh[:, 0:k], 0.0, whr_y, op0=AO.max, op1=AO.mult
            )
            nc.vector.scalar_tensor_tensor(
                union, bc_hi[:, 2 * k : 3 * k], ca_t[:, 4:5], inter,
                op0=AO.add, op1=AO.subtract,
            )
            nc.vector.reciprocal_approx_fast(out=recip, in_=union)
            nc.vector.tensor_tensor(iou, inter, recip, op=AO.mult)
            import concourse.mybir as mb
            nc.vector.tensor_reduce(
                rowmax, iou, axis=mb.AxisListType.X, op=AO.max
            )
            s_ps = ps.tile([1, 1], f32, tag="gt")
            nc.tensor.matmul(s_ps, rowmax, ones_col[0:k, :])
            nc.scalar.mul(out_sb, s_ps[0:1, 0:1], 1.0 / float(k))
            nc.sync.dma_start(out=out_d.ap(), in_=out_sb)

    nc.compile()
    return nc


def _get(k):
    if k not in _CACHE:
        _CACHE[k] = build(k)
    return _CACHE[k]


def kernel(pred, target, k):
    k = int(k)
    nc = _get(k)
    from concourse.bass_utils import run_bass_kernel_spmd

    pred = np.asarray(pred, np.float32).reshape(C, HW)
    target = np.asarray(target, np.float32).reshape(C, HW)
    conf = np.ascontiguousarray(pred[0 : C : 19, :]).reshape(-1)  # 640 raw
    in_map = {
        "vp": np.ascontiguousarray(_host_arrange(pred).reshape(P, NCHUNK * 7)),
        "vt": np.ascontiguousarray(
            _host_arrange(target).reshape(P, NCHUNK * 7)
        ),
        "rba": np.ascontiguousarray(
            np.broadcast_to(conf[None, 0 : NCELL // 2], (P, NCELL // 2))
        ),
        "rbb": np.ascontiguousarray(
            np.broadcast_to(conf[None, NCELL // 2 :], (P, NCELL // 2))
        ),
    }
    res = run_bass_kernel_spmd(nc, [dict(in_map) for _ in range(8)],
                               core_ids=list(range(8)))
    return np.float32(res.results[0]["out"][0, 0])
